# revision 30
# baseline (speedup 1.0000x reference)
"""Causal multi-head attention with RoPE on 8 Trainium2 NeuronCores.

Reference computation (fp32):
    qkv = x @ Wqkv.T ; split q,k,v ; heads 16 x 64 ; interleaved-pair RoPE on
    q,k ; causal softmax(q k^T / 8) @ v ; concat heads ; out @ Wout.T

Sharding: core c -> batch b=c//2, head-group g=c%2 (heads 8g..8g+8).
Each core computes a [2048, 1024] partial of the output projection for its
batch (contraction over its 512 head-dims); host sums core pairs (bf16
partials, fp32 accumulate on host).

Kernel-internal layout tricks (software-pipelined):
  - Wqkv rows per head are permuted 16-interleaved (evens/odds of pair
    blocks) so the RoPE pair-swap becomes a within-quadrant DVE
    stream_shuffle. The same permutation applied to q and k leaves q.k^T
    invariant.
  - Scores are computed transposed (S^T[k, q]) so the PV matmul needs no
    transposes; both heads of a pair share one 2-bank PSUM tile so a single
    FD=1024 activation exponentiates them together.
  - The causal mask on diagonal 128-blocks is applied AFTER the exp by a
    gpsimd affine_select (fill strictly-upper triangle with 0) - neither the
    PE nor the DVE touches masks.
  - PV is causally trimmed: for diagonal key-tiles only columns [lo:512]
    are accumulated, with region-wise stop flags.
  - PV uses a ones-augmented V (ones in stationary col 0, head dims at
    col 64:128) so PSUM partition 0 accumulates the softmax denominator for
    free; a fast approx DVE reciprocal + gpsimd partition_broadcast turns it
    into a [64, 512] fp32 divisor tile, and the evacuation of the PV psum
    is fused with the division (one DVE multiply).
  - The whole kernel is software-pipelined at emission level: QKV matmuls
    of head-pair hp+1 (and the output projection, for the last pair) are
    interleaved into the attention loop of head-pair hp so the PE never
    idles long enough for the HAM clock gate to re-throttle it.

Matmul dtype MM_DT (env): bfloat16 (default, host pre-rounds inputs),
float32r, or float32. The softmax denominator / division chain is fp32->bf16.
"""

import math
import os
import sys

import numpy as np

sys.path.insert(0, "/opt/trn_rl_repo")

import concourse.bass as bass  # noqa: E402,F401  (re-exported for tooling)
import concourse.mybir as mybir  # noqa: E402
from concourse import bacc, library_config, tile  # noqa: E402
from concourse.masks import make_identity  # noqa: E402

D_MODEL = 1024
NUM_HEADS = 16
DH = 64
S = 2048
B = 4
THETA = 10000.0
P = 128
N_CORES = 8
F = 512  # free-dim chunk
N_SC = S // F  # 4 s-chunks
N_QT = S // P  # 16 q-tiles of 128
HPAIRS = 4  # head pairs per core
LOOKAHEAD = 3

MM_DT = getattr(mybir.dt, os.environ.get("MM_DT", "bfloat16"))
PIPE = os.environ.get("PIPE", "1") == "1"
DOFF = 64   # PV dims partition offset in psum (64-partition reads need it)
VW = DOFF + 64  # V stationary width: [ones | dead | 64 dims]
SHUF16 = [(i + 16) % 32 for i in range(32)]  # swap 16-blocks in a quadrant


class _Filler:
    """Queue of emission-step generators, pulled into the attention loop."""

    def __init__(self):
        self.gens = []

    def add(self, gen):
        self.gens.append(gen)

    def pull(self, n=1):
        while n > 0 and self.gens:
            try:
                next(self.gens[0])
                n -= 1
            except StopIteration:
                self.gens.pop(0)

    def drain(self):
        while self.gens:
            self.pull(1 << 20)


def build_program(debug: bool = False):
    """Build the single-core SPMD program (identical on all 8 cores)."""
    nc = bacc.Bacc("TRN2", target_bir_lowering=False, debug=debug,
                   enable_asserts=debug)
    f32 = mybir.dt.float32
    cdt = MM_DT

    xt_d = nc.dram_tensor("xt", [D_MODEL, S], cdt, kind="ExternalInput")
    wq_d = nc.dram_tensor("wqkv", [D_MODEL, 12 * P], cdt, kind="ExternalInput")
    wo_d = nc.dram_tensor("wout", [4 * P, D_MODEL], cdt, kind="ExternalInput")
    cos_d = nc.dram_tensor("costab", [P, S], cdt, kind="ExternalInput")
    sinw_d = nc.dram_tensor("sinswt", [P, S], cdt, kind="ExternalInput")
    out_d = nc.dram_tensor("out", [S, D_MODEL], cdt, kind="ExternalOutput")

    xt_r = xt_d.ap().rearrange("(dc p) s -> p dc s", p=P)  # [128, 8, 2048]
    wq_r = wq_d.ap().rearrange("(dc p) n -> p dc n", p=P)  # [128, 8, 1536]
    wo_r = wo_d.ap().rearrange("(hp p) e -> p hp e", p=P)  # [128, 4, 1024]

    with tile.TileContext(nc) as tc:
        with (
            tc.tile_pool(name="const", bufs=1) as const,
            tc.tile_pool(name="qkv", bufs=3) as qkvp,
            tc.tile_pool(name="tmp", bufs=2) as tmpp,
            tc.tile_pool(name="outt", bufs=1) as outtp,
            tc.tile_pool(name="exp", bufs=7) as expp,
            tc.tile_pool(name="den", bufs=3) as denp,
            tc.tile_pool(name="fin", bufs=3) as finp,
            tc.tile_pool(name="psS", bufs=2, space="PSUM") as psS,
            tc.tile_pool(name="psQ", bufs=2, space="PSUM") as psQ,
            tc.tile_pool(name="psP", bufs=2, space="PSUM") as psP,
        ):
            # ---- constants / input DMAs (issued in consumption order) ----
            # gpsimd ucode load for partition_broadcast, FIRST: reloading
            # later races with native gpsimd ops already in flight
            nc.gpsimd.load_library(library_config.attn)
            ident = const.tile([P, P], f32, name="ident")
            make_identity(nc, ident)
            identc = const.tile([P, P], cdt, name="identc")
            nc.vector.tensor_copy(identc[:], ident[:])

            wqall = const.tile([P, 8, 12 * P], cdt, name="wqall")
            xts = const.tile([P, 8, S], cdt, name="xts")
            cost = const.tile([P, S], cdt, name="cost")
            sinw = const.tile([P, S], cdt, name="sinw")
            woutt = const.tile([P, 4, D_MODEL], cdt, name="woutt")
            # issue input DMAs from three idle queues in parallel so the
            # per-issue cost (~0.65us) does not serialize the fill
            for dc in range(8):
                nc.sync.dma_start(wqall[:, dc, :], wq_r[:, dc, :])
                nc.scalar.dma_start(xts[:, dc, 0:S // 2],
                                    xt_r[:, dc, 0:S // 2])
                if dc == 3:
                    nc.gpsimd.dma_start(cost[:], cos_d.ap())
                    nc.gpsimd.dma_start(sinw[:], sinw_d.ap())
            for dc in range(8):
                nc.scalar.dma_start(xts[:, dc, S // 2:S],
                                    xt_r[:, dc, S // 2:S])
            nc.gpsimd.dma_start(woutt[:], wo_r)

            # attention output (d-major), all 4 head pairs: rows=[hA|hB] dims
            outt = outtp.tile([P, HPAIRS, S], cdt, name="outt")

            state = {}

            def qkv_steps(hp):
                """Generator: one `yield` per schedulable emission step."""
                st = {}
                state[hp] = st
                st["q_rot"] = qkvp.tile([P, S], cdt, tag="q_rot",
                                        name="q_rot")
                st["k_rot"] = qkvp.tile([P, S], cdt, tag="k_rot",
                                        name="k_rot")
                v_sb = qkvp.tile([P, 2, N_QT, VW], cdt, tag="v_sb",
                                 name="v_sb")
                st["v_sb"] = v_sb
                for h2 in (0, 1):
                    # ones column FIRST so the PV denominator lands in PSUM
                    # partition 0 (custom-DVE reciprocal needs offset 0);
                    # head dims live in cols DOFF:DOFF+64 (PSUM reads must
                    # start at a 32-aligned partition). Cols 1:DOFF are dead.
                    nc.vector.memset(v_sb[:, h2, :, 0:1], 1.0)
                    nc.gpsimd.memset(v_sb[:, h2, :, 1:DOFF], 0.0)
                yield
                # q and k groups (d-major), HALF-MAJOR: both groups' first
                # S-half (matmuls + stream_shuffle RoPE) are emitted before
                # either group's second half, so attention on the first two
                # s-chunks can start while the rest of x still streams in.
                st["rope_ready"] = 0
                raws = {}

                def qk_half(half):
                    for gi, key in ((0, "q_rot"), (1, "k_rot")):
                        if half == 0:
                            raws[gi] = qkvp.tile([P, S], cdt,
                                                 tag=f"raw{gi}",
                                                 name=f"raw{gi}")
                        raw = raws[gi]
                        for sc in (2 * half, 2 * half + 1):
                            sl = slice(sc * F, (sc + 1) * F)
                            ps = psQ.tile([P, F], f32, tag="q", name="psw")
                            for dc in range(8):
                                nc.tensor.matmul(
                                    ps,
                                    wqall[:, dc,
                                          hp * 3 * P + gi * P:
                                          hp * 3 * P + (gi + 1) * P],
                                    xts[:, dc, sl],
                                    start=(dc == 0), stop=(dc == 7),
                                )
                            if hp == 0:
                                nc.scalar.copy(raw[:, sl], ps)
                            else:
                                nc.vector.tensor_copy(raw[:, sl], ps)
                            yield
                        # rot = raw*cos + swap16(raw)*sins on this S-half
                        # (rows 16-interleaved by the host perm, so the
                        # pair-swap is a within-quadrant stream_shuffle)
                        rot = st[key]
                        hs = slice(2 * half * F, (2 * half + 2) * F)
                        tcs = tmpp.tile([P, S // 2], cdt, tag="tcs",
                                        name="tcs", bufs=3)
                        nc.vector.tensor_tensor(
                            tcs[:, :], raw[:, hs], cost[:, hs],
                            mybir.AluOpType.mult)
                        yield
                        shf = tmpp.tile([P, S // 2], cdt, tag="shf",
                                        name="shf", bufs=3)
                        nc.vector.stream_shuffle(shf[:, :], raw[:, hs],
                                                 SHUF16)
                        yield
                        nc.vector.tensor_tensor(
                            rot[:, hs], shf[:, :], sinw[:, hs],
                            mybir.AluOpType.mult)
                        yield
                        nc.vector.tensor_tensor(
                            rot[:, hs], rot[:, hs], tcs[:, :],
                            mybir.AluOpType.add)
                        yield
                    st["rope_ready"] = half + 1

                for _s in qk_half(0):
                    yield

                # v group: d-major matmul, then PE-transpose to s-major.
                # attention for this pair may start after the first s-chunk
                # of V lands (it covers key-tiles 0..3); the rest trails as
                # filler steps.
                for sc in range(N_SC):
                    if sc == 1:
                        # second q/k half (+ rope) before the remaining V
                        for _s in qk_half(1):
                            yield
                    sl = slice(sc * F, (sc + 1) * F)
                    ps = psQ.tile([P, F], f32, tag="q", name="pswv")
                    for dc in range(8):
                        nc.tensor.matmul(
                            ps, wqall[:, dc,
                                      hp * 3 * P + 2 * P:hp * 3 * P + 3 * P],
                            xts[:, dc, sl], start=(dc == 0), stop=(dc == 7),
                        )
                    vdm = tmpp.tile([P, F], cdt, tag="vdm", name="vdm")
                    if hp == 0:
                        nc.scalar.copy(vdm[:], ps)
                    else:
                        nc.vector.tensor_copy(vdm[:], ps)
                    yield
                    for jh in range(2):
                        for j in (2 * jh, 2 * jh + 1):
                            kt = sc * 4 + j
                            pt = psQ.tile([P, P], cdt, tag="q", name="pt")
                            nc.tensor.transpose(pt[:, 0:P],
                                                vdm[:, j * P:(j + 1) * P],
                                                identc[:])
                            nc.vector.tensor_copy(
                                v_sb[:, 0, kt, DOFF:DOFF + 64], pt[:, 0:64])
                            nc.vector.tensor_copy(
                                v_sb[:, 1, kt, DOFF:DOFF + 64], pt[:, 64:128])
                        yield
                    st["v_ready"] = sc + 1
                    if sc == 0:
                        st["done"] = True

            def proj_steps(qc):
                """Output projection for s-tiles of chunk qc + DMA out."""
                for sti in range(4 * qc, 4 * qc + 4):
                    for ec in range(2):
                        esl = slice(ec * F, (ec + 1) * F)
                        pf = psQ.tile([P, F], f32, tag="q", name="pfw")
                        for hp in range(HPAIRS):
                            nc.tensor.matmul(
                                pf, outt[:, hp, sti * P:(sti + 1) * P],
                                woutt[:, hp, esl],
                                start=(hp == 0), stop=(hp == 3),
                            )
                        fo = finp.tile([P, F], cdt, tag="fo", name="fo")
                        nc.vector.tensor_copy(fo[:], pf)
                        nc.sync.dma_start(
                            out_d.ap()[sti * P:(sti + 1) * P, esl], fo[:])
                        yield

            def proj_tile(sti):
                """Projection + store for one 128-row s-tile."""
                for ec in range(2):
                    esl = slice(ec * F, (ec + 1) * F)
                    pf = psQ.tile([P, F], f32, tag="q", name="pfw")
                    for hp in range(HPAIRS):
                        nc.tensor.matmul(
                            pf, outt[:, hp, sti * P:(sti + 1) * P],
                            woutt[:, hp, esl],
                            start=(hp == 0), stop=(hp == 3),
                        )
                    fo = finp.tile([P, F], cdt, tag="fo", name="fo")
                    nc.vector.tensor_copy(fo[:], pf)
                    nc.sync.dma_start(
                        out_d.ap()[sti * P:(sti + 1) * P, esl], fo[:])

            def attn(hp, filler, on_qc_done=None, qc_order=None,
                     tail_regions=False):
                """Causal attention for head pair hp, pulling filler steps."""
                st = state[hp]
                q_rot, k_rot, v_sb = st["q_rot"], st["k_rot"], st["v_sb"]
                for qc in (qc_order if qc_order is not None
                           else range(N_SC)):
                    # q/k rope for this chunk's columns must be emitted
                    # (program order = dependency order)
                    need = 1 if qc < 2 else 2
                    while st.get("rope_ready", 0) < need:
                        if not filler.gens:
                            raise RuntimeError("rope half not emitted")
                        filler.pull(1)
                    qsl = slice(qc * F, (qc + 1) * F)
                    po = [psP.tile([P, F], f32, tag="po", name=f"po{h2}")
                          for h2 in range(2)]
                    nkt = 4 * qc + 4

                    def emit_scores(kt):
                        lo = max(0, (kt - 4 * qc) * P)
                        # both heads in one 2-bank psum tile; disjoint PE
                        # row groups (0:64 / 64:128) run concurrently
                        sp = psS.tile([P, 2, F], f32, tag="sp", name="sp")
                        for h2 in (0, 1):
                            base = 64 * h2
                            nc.tensor.matmul(
                                sp[:, h2, lo:F],
                                k_rot[base:base + 64, kt * P:(kt + 1) * P],
                                q_rot[base:base + 64,
                                      qc * F + lo:(qc + 1) * F],
                                start=True, stop=True,
                            )
                        return sp

                    def emit_exp(kt, sp):
                        lo = max(0, (kt - 4 * qc) * P)
                        ex = expp.tile([P, 2, F], cdt, name="ex")
                        nc.scalar.activation(
                            ex[:, :, lo:F], sp[:, :, lo:F],
                            mybir.ActivationFunctionType.Exp,
                            scale=1.0 / math.sqrt(DH))
                        if kt >= 4 * qc:
                            # zero the strictly-upper triangle of the
                            # transposed diagonal 128-block (keys > q)
                            nc.gpsimd.affine_select(
                                out=ex[:, :, lo:lo + P],
                                in_=ex[:, :, lo:lo + P],
                                compare_op=mybir.AluOpType.is_ge,
                                fill=0.0, base=0,
                                pattern=[[0, 2], [1, P]],
                                channel_multiplier=-1,
                            )
                        return ex

                    def emit_pv(kt, ex):
                        j = kt - 4 * qc
                        for h2 in (0, 1):
                            if j < 0:
                                nc.tensor.matmul(
                                    po[h2][0:VW, :],
                                    v_sb[:, h2, kt, 0:VW],
                                    ex[:, h2, :],
                                    start=(kt == 0), stop=False,
                                    skip_group_check=True,
                                )
                            else:
                                lo = j * P
                                # region [lo:lo+128] sees its last
                                # contribution here; [lo+128:512] continues
                                nc.tensor.matmul(
                                    po[h2][0:VW, lo:lo + P],
                                    v_sb[:, h2, kt, 0:VW],
                                    ex[:, h2, lo:lo + P],
                                    start=(kt == 0), stop=True,
                                    skip_group_check=True,
                                )
                                if lo + P < F:
                                    nc.tensor.matmul(
                                        po[h2][0:VW, lo + P:F],
                                        v_sb[:, h2, kt, 0:VW],
                                        ex[:, h2, lo + P:F],
                                        start=(kt == 0), stop=False,
                                        skip_group_check=True,
                                    )

                    pend = {kt: emit_scores(kt)
                            for kt in range(min(LOOKAHEAD, nkt))}
                    for kt in range(nkt):
                        if kt + LOOKAHEAD < nkt:
                            pend[kt + LOOKAHEAD] = emit_scores(kt + LOOKAHEAD)
                        ex = emit_exp(kt, pend.pop(kt))
                        # the V s-chunk covering this key-tile must have been
                        # emitted (program order = dependency order)
                        while st.get("v_ready", 0) <= kt // 4:
                            if not filler.gens:
                                raise RuntimeError("v_sb chunk not emitted")
                            filler.pull(1)
                        emit_pv(kt, ex)
                        if tail_regions and qc == 0:
                            # trailing chunk: every key-tile is diagonal, so
                            # column region [kt*128,(kt+1)*128) is complete
                            # now - divide it and project it immediately
                            rsl = slice(kt * P, (kt + 1) * P)
                            for h2 in range(2):
                                rcr = denp.tile([1, P], f32, tag="rcr",
                                                bufs=3, name="rcr")
                                nc.vector.reciprocal_approx_fast(
                                    rcr[:], po[h2][0:1, rsl])
                                pbr = denp.tile([64, P], f32, tag="pbr",
                                                bufs=3, name="pbr")
                                nc.gpsimd.partition_broadcast(
                                    pbr[:], rcr[:], channels=64)
                                nc.vector.tensor_tensor(
                                    outt[64 * h2:64 * h2 + 64, hp,
                                         qc * F + kt * P:
                                         qc * F + (kt + 1) * P],
                                    po[h2][DOFF:DOFF + 64, rsl], pbr[:],
                                    mybir.AluOpType.mult)
                            # project the PREVIOUS region (its divide chain
                            # has drained) so this one's chain overlaps the
                            # next pv instead of blocking it
                            if kt > 0:
                                proj_tile(4 * qc + kt - 1)
                            if kt == nkt - 1:
                                proj_tile(4 * qc + kt)
                            continue
                        if kt < nkt - 2:
                            filler.pull(1)

                    if tail_regions and qc == 0:
                        continue
                    # evac + divide, fused: approx DVE reciprocal of the
                    # ones-row (fp32), gpsimd partition-broadcast to 64
                    # rows, one DVE multiply fused with the evacuation
                    rcs = []
                    for h2 in range(2):
                        rcf = denp.tile([1, F], f32, tag="rcf", bufs=3,
                                        name="rcf")
                        nc.vector.reciprocal_approx_fast(rcf[:],
                                                         po[h2][0:1, :])
                        rcs.append(rcf)
                    filler.pull(1)  # PE work while the reciprocals run
                    pbws = []
                    for h2 in range(2):
                        pbw = denp.tile([64, F], f32, tag="pbw", bufs=3,
                                        name="pbw")
                        nc.gpsimd.partition_broadcast(pbw[:], rcs[h2][:],
                                                      channels=64)
                        pbws.append(pbw)
                    for h2 in range(2):
                        nc.vector.tensor_tensor(
                            outt[64 * h2:64 * h2 + 64, hp, qsl],
                            po[h2][DOFF:DOFF + 64, :], pbws[h2][:],
                            mybir.AluOpType.mult)
                    if on_qc_done is not None:
                        on_qc_done(qc)

            # ---- schedule: qkv(0); attn(hp) || qkv(hp+1..)/proj ----
            filler = _Filler()
            if PIPE:
                g0, g1 = qkv_steps(0), qkv_steps(1)
                alive = {0: True, 1: True}

                def _step(g, i):
                    try:
                        next(g)
                    except StopIteration:
                        alive[i] = False

                tick = 0
                while alive[0] and not state.get(0, {}).get("done"):
                    _step(g0, 0)
                    if tick % 2 == 1 and alive[1]:
                        _step(g1, 1)
                    tick += 1
                if alive[0]:
                    filler.add(g0)
                if alive[1]:
                    filler.add(g1)
                filler.add(qkv_steps(2))
                for hp in range(HPAIRS):
                    # qkv(hp) must be fully emitted before attn(hp) reads it
                    while not state.get(hp, {}).get("done"):
                        filler.pull(1)
                    if hp == 1:
                        filler.add(qkv_steps(3))
                    if hp + 1 < HPAIRS:
                        attn(hp, filler)
                    else:
                        # last pair: run s-chunks big-to-small and feed each
                        # finished chunk's projection back into the loop, so
                        # only the smallest chunk's projection trails
                        tr = os.environ.get("TAILR", "0") == "1"
                        attn(hp, filler,
                             on_qc_done=lambda qc: (
                                 filler.add(proj_steps(qc))
                                 if (qc != 0 or not tr) else None),
                             qc_order=[3, 2, 1, 0], tail_regions=tr)
                filler.drain()
            else:
                for hp in range(HPAIRS):
                    for _ in qkv_steps(hp):
                        pass
                    attn(hp, filler)
                for qc in range(N_SC):
                    filler.add(proj_steps(qc))
                filler.drain()

    nc.compile()
    return nc


def _rope_tables():
    k = np.arange(DH // 2, dtype=np.float64)
    invf = THETA ** (-2.0 * k / DH)
    pos = np.arange(S, dtype=np.float64)
    # row r (within a 64-row head block, 16-interleaved): pair index
    # 16*(r//32) + r%16; even slots (r%32<16) carry -sin, odd slots +sin
    r = np.arange(DH)
    pair = 16 * (r // 32) + (r % 16)
    sign = np.where((r % 32) < 16, -1.0, 1.0)
    ang = invf[pair][:, None] * pos[None, :]  # [64, S]
    cos64 = np.cos(ang)
    sin64 = sign[:, None] * np.sin(ang)
    cos = np.tile(cos64, (2, 1)).astype(np.float32)          # [128, S]
    sins = np.tile(sin64, (2, 1)).astype(np.float32)
    return cos, sins


def _np_dt():
    if MM_DT == mybir.dt.bfloat16:
        import ml_dtypes
        return np.dtype(ml_dtypes.bfloat16)
    return np.dtype(np.float32)


def host_inputs(x, Wqkv, Wout, core):
    """Per-core input dict (cast to the compute dtype on host)."""
    ndt = _np_dt()
    b, g = core // 2, core % 2
    xt = np.ascontiguousarray(x[b].T).astype(ndt)  # [1024, 2048]
    perm = np.concatenate([
        np.arange(0, 32, 2), np.arange(1, 32, 2),    # pairs 0..15
        np.arange(32, 64, 2), np.arange(33, 64, 2),  # pairs 16..31
    ])
    blocks = []
    for hp in range(HPAIRS):
        hA = 8 * g + 2 * hp
        for off, do_perm in ((0, True), (D_MODEL, True), (2 * D_MODEL, False)):
            for h in (hA, hA + 1):
                rows = Wqkv[off + h * DH: off + (h + 1) * DH]
                if do_perm:
                    rows = rows[perm]
                blocks.append(rows)
    wq = np.ascontiguousarray(np.concatenate(blocks, 0).T).astype(ndt)
    wo = np.ascontiguousarray(Wout[:, 512 * g:512 * (g + 1)].T).astype(ndt)
    cos, sins = _rope_tables()
    return {"xt": xt, "wqkv": wq, "wout": wo,
            "costab": cos.astype(ndt), "sinswt": sins.astype(ndt)}


_CACHE = {}


def kernel(x, Wqkv, Wout):
    from concourse.bass_utils import run_bass_kernel_spmd

    x = np.asarray(x, dtype=np.float32)
    Wqkv = np.asarray(Wqkv, dtype=np.float32)
    Wout = np.asarray(Wout, dtype=np.float32)

    if "nc" not in _CACHE:
        _CACHE["nc"] = build_program(debug=False)
    nc = _CACHE["nc"]

    in_maps = [host_inputs(x, Wqkv, Wout, c) for c in range(N_CORES)]
    res = run_bass_kernel_spmd(nc, in_maps, list(range(N_CORES))).results
    out = np.empty((B, S, D_MODEL), dtype=np.float32)
    for b in range(B):
        out[b] = (res[2 * b]["out"].astype(np.float32)
                  + res[2 * b + 1]["out"].astype(np.float32))
    return out
